# revision 29
# baseline (speedup 1.0000x reference)
"""Trainium2 Bass kernel for nn_AudioSNN: 2-layer spiking NN (snntorch Leaky).

Reference semantics per timestep t (over T=200 steps):
    cur1 = x_t @ w1.T + b1                      # [B, 128]
    m1   = 0.9*m1 + cur1 - (m1_prev > 1)        # reset-by-subtract
    spk1 = (m1 > 1)
    cur2 = spk1 @ w2.T + b2                     # [B, 5]
    m2   = 0.9*m2 + cur2 - (m2_prev > 1)
    out[t] = spk2 = (m2 > 1)

Strategy (pure data-parallel over batch, 8 cores x 1024 batch rows):
  - Transposed layout: states kept as [feature, batch] so H=128 sits on
    SBUF partitions and batch on the free dim.
  - One fused custom DVE op does a whole membrane update in a single
    instruction:  m_new = m*beta - (m > thr) + cur + bias.
  - Spikes are encoded via ACT Sign: sg = sign(1 - m1) = -sign(m1 - 1),
    so spk1 = (1 - sg)/2.  Layer-2 matmul uses lhsT ~ -0.5*w2.T and a
    per-partition bias to reconstruct w2 @ spk1.
  - All matmuls run in fp16 with hi/lo split pairs (x = xh + xl exactly
    to ~2^-22 rel; w likewise), accumulated exactly in fp32 PSUM:
    mm1 = wh@xh + wh@xl + wl@xh (3 passes, row-tiled over two K=40 row
    groups); mm2 = w2h@sg + w2l@sg (2 passes, col-tiled 4x).  fp16
    streams 1 cyc/row on the PE vs 4 cyc/row for fp32.
  - x is DMA'd in XB-step batches; m2 is accumulated in a wide staging
    tile and written out as 4 large DMAs every OB steps; host applies
    the (m2 > 1) threshold.
"""

import numpy as np

import concourse.bacc as bacc
import concourse.mybir as mybir
import concourse.tile as tile
import concourse.dve_ops as dve_ops
from concourse.dve_ops import DveOp
from concourse.dve_spec import Spec, Src0, Src1, C0, C1, C2, lower as dve_lower
from concourse.dve_uop import DveOpSpec
from concourse.bass_utils import run_bass_kernel_spmd

F32 = mybir.dt.float32
F16 = mybir.dt.float16

B, T, F, H, C = 8192, 200, 40, 128, 5
NCORES = 8
BL = B // NCORES          # 1024 batch rows per core
BH = BL // 2              # 512 per mm1 row-group
BETA, THR = 0.9, 1.0
NG = 4                    # col-tile groups for layer 2
BG = BL // NG             # 256 batch rows per col group
XB = 4                    # timesteps per x DMA batch
OB = 8                    # timesteps per output DMA batch


# --------------------------------------------------------------------------
# Custom DVE op: fused SNN membrane update
# --------------------------------------------------------------------------

def _snn_ref(in0, in1, s0, s1, imm2):
    out = (
        in0.astype(np.float32) * imm2
        - (in0 > s1).astype(np.float32)
        + in1.astype(np.float32)
        + s0
    )
    return out.astype(np.float32)


def _register_snn_op() -> DveOp:
    """out = in0*imm2 - (in0 > s1) + in1 + s0"""
    name = "SNN_MEMBRANE_STEP"
    for op in dve_ops.OPS:
        if op.name == name:
            return op
    body = Src0 * C2 - (Src0 > C1) + Src1 + C0
    spec = Spec(body=body, reference=_snn_ref)
    shas = {}
    for ver in ("v3", "v4"):
        uops = dve_lower(spec, ver=ver)
        shas[ver] = DveOpSpec(name=name, opcode=0, uops=uops, rd1_en=True).sha(ver)
    op = DveOp(name, spec, subdim=False, uops_sha=shas)
    dve_ops.OPS.append(op)
    dve_ops._SUB_OPCODE_FOR_NAME[op.name] = (
        dve_ops._CUSTOM_DVE_ROW_BASE + len(dve_ops.OPS) - 1
    )
    dve_ops.CUSTOM_DVE_SPECS[op.name] = spec
    return op


SNN_OP = _register_snn_op()


# --------------------------------------------------------------------------
# Bass module
# --------------------------------------------------------------------------

def build_module(t_steps: int = T, probe: str = ""):
    assert t_steps % (XB * OB // np.gcd(XB, OB)) == 0 or t_steps % XB == 0
    assert t_steps % XB == 0 and t_steps % OB == 0
    tb = t_steps // XB
    ob_n = t_steps // OB
    nc = bacc.Bacc("TRN2", target_bir_lowering=False, debug=False)

    # x packed for the K-stacked 3-pass mm1: rows 0-39 = xh, rows 40-79
    # = xl, rows 80-119 = xh again (pairs with [wh; wh; wl] on the weight
    # side).  XB steps side by side in the free dim.
    XW = XB * BL
    xq = nc.dram_tensor("xq", [tb, 120, XW], F16, kind="ExternalInput").ap()
    # w1 fp16 triple-K stack [wh; wh; wl]
    w1trip = nc.dram_tensor("w1trip", [120, H], F16, kind="ExternalInput").ap()
    # w2 fp16 pair (padded to 32 cols)
    w2qh = nc.dram_tensor("w2qh", [H, 32], F16, kind="ExternalInput").ap()
    w2ql = nc.dram_tensor("w2ql", [H, 32], F16, kind="ExternalInput").ap()
    bias1 = nc.dram_tensor("bias1", [H, 1], F32, kind="ExternalInput").ap()
    bias2 = nc.dram_tensor("bias2", [128, 1], F32, kind="ExternalInput").ap()
    # out[g, c, t, j] = m2 for class c, batch b = g*BG + j at step t
    out = nc.dram_tensor(
        "out", [NG, C, t_steps, BG], F32, kind="ExternalOutput"
    ).ap()

    with tile.TileContext(nc) as tc:
        with (
            tc.tile_pool(name="const", bufs=1) as cpool,
            tc.tile_pool(name="state", bufs=1) as spool,
            tc.tile_pool(name="xin", bufs=6) as xpool,
            tc.tile_pool(name="sgn", bufs=6) as gpool,
            tc.tile_pool(name="stage", bufs=4) as stpool,
            tc.tile_pool(name="ps1", bufs=3, space="PSUM") as p1pool,
            tc.tile_pool(name="ps2", bufs=2, space="PSUM") as p2pool,
        ):
            w1t_s = cpool.tile([120, H], F16)
            w2qh_s = cpool.tile([H, 32], F16)
            w2ql_s = cpool.tile([H, 32], F16)
            b1_s = cpool.tile([H, 1], F32)
            b2_s = cpool.tile([128, 1], F32)
            nc.sync.dma_start(w1t_s[:], w1trip[:])
            nc.sync.dma_start(w2qh_s[:], w2qh[:])
            nc.sync.dma_start(w2ql_s[:], w2ql[:])
            nc.sync.dma_start(b1_s[:], bias1[:])
            nc.sync.dma_start(b2_s[:], bias2[:])

            m1_pool_prev = spool.tile([H, BL], F32, tag="m1a")
            nc.gpsimd.memset(m1_pool_prev[:], 0.0)
            m1_pool_alt = spool.tile([H, BL], F32, tag="m1b")
            m1_pool_alt2 = spool.tile([H, BL], F32, tag="m1c")
            m1_bufs = [m1_pool_alt, m1_pool_alt2, m1_pool_prev]
            m1_prev = m1_pool_prev
            p1_st = p2_st = x_st = sg_st = None
            if probe == "no_mm1":
                p1_st = spool.tile([H, BL], F32, tag="p1s")
                nc.gpsimd.memset(p1_st[:], 0.1)
            if probe == "no_mm2":
                p2_st = spool.tile([128, BG], F32, tag="p2s")
                nc.gpsimd.memset(p2_st[:], 0.1)
            if probe == "no_xdma":
                x_st = spool.tile([120, XW], F16, tag="xs")
                nc.sync.dma_start(x_st[:], xq[0])
            if probe == "no_act":
                sg_st = spool.tile([H, BL], F16, tag="sgs")
                nc.gpsimd.memset(sg_st[:], 1.0)
            stage_prev = stpool.tile([128, OB * BG], F32, tag="st")
            nc.gpsimd.memset(stage_prev[:, (OB - 1) * BG :], 0.0)
            state = {
                "m2_prev": stage_prev[:, (OB - 1) * BG : OB * BG],
                "stage": None,
                "p2": None,
            }

            def l2_step(tau):
                """Membrane-2 update + output for step tau (runs one step
                late so the DVE queue never stalls on the ACT->PE chain)."""
                i2 = tau % OB
                if i2 == 0:
                    state["stage"] = stpool.tile([128, OB * BG], F32, tag="st", name="stg")
                m2 = state["stage"][:, i2 * BG : (i2 + 1) * BG]
                if probe != "no_dve":
                    nc.vector._custom_dve(
                        SNN_OP, out=m2, in0=state["m2_prev"], in1=state["p2"][:],
                        s0=b2_s[:, 0:1], s1=THR, imm2=BETA,
                    )
                state["m2_prev"] = m2
                if i2 == OB - 1 and probe != "no_outdma":
                    ob = tau // OB
                    for g in range(NG):
                        nc.sync.dma_start(
                            out[g, :, ob * OB : (ob + 1) * OB, :].rearrange(
                                "c t j -> c (t j)"
                            ),
                            state["stage"][32 * g : 32 * g + C, :],
                        )

            for t in range(t_steps):
                k, s = divmod(t, XB)

                if s == 0:
                    if probe == "no_xdma":
                        xt = x_st
                    else:
                        xt = xpool.tile([120, XW], F16, tag="x")
                        nc.sync.dma_start(xt[:], xq[k])

                # mm1: cur1 = w1 @ x via one K=120 stacked pass
                # ([wh; wh; wl] . [xh; xl; xh]), split in two N=512 halves
                p1 = p1_st if probe == "no_mm1" else p1pool.tile([H, BL], F32, tag="p1")
                if probe != "no_mm1":
                    for half in (0, BH):
                        nc.tensor.matmul(
                            p1[:, half : half + BH],
                            w1t_s[:],
                            xt[:, s * BL + half : s * BL + half + BH],
                            start=True, stop=True,
                        )

                # m1 = beta*m1 - (m1 > 1) + cur1 + b1  (ping-pong buffers
                # so the next step's write doesn't WAR-wait on ACT's read)
                m1 = m1_bufs[t % 3]
                if probe != "no_dve":
                    nc.vector._custom_dve(
                        SNN_OP, out=m1[:], in0=m1_prev[:], in1=p1[:],
                        s0=b1_s[:, 0:1], s1=THR, imm2=BETA,
                    )
                m1_prev = m1

                # sg = sign(1 - m1)  (= -sign(m1-1); spk1 = (1 - sg)/2)
                if probe == "no_act":
                    sg = sg_st
                else:
                    sg = gpool.tile([H, BL], F16, tag="sg")
                    nc.scalar.activation(
                        sg[:], m1[:], mybir.ActivationFunctionType.Sign,
                        bias=1.0, scale=-1.0,
                    )

                # cur2: p2[32g+c, j] = -0.5*(w2 @ sgn1)[c, 256g+j], 2-pass
                p2 = p2_st if probe == "no_mm2" else p2pool.tile([128, BG], F32, tag="p2")
                for g in () if probe == "no_mm2" else range(NG):
                    gs = sg[:, BG * g : BG * (g + 1)]
                    nc.tensor.matmul(
                        p2[32 * g : 32 * (g + 1), :], w2qh_s[:], gs,
                        start=True, stop=False, tile_position=(0, 32 * g),
                    )
                    nc.tensor.matmul(
                        p2[32 * g : 32 * (g + 1), :], w2ql_s[:], gs,
                        start=False, stop=True, tile_position=(0, 32 * g),
                    )

                # m2(t-1) update, one step behind
                if t > 0:
                    l2_step(t - 1)
                state["p2"] = p2

            l2_step(t_steps - 1)

    nc.compile()
    return nc


_MODULE_CACHE: dict = {}


def _get_module(t_steps: int = T):
    if t_steps not in _MODULE_CACHE:
        _MODULE_CACHE[t_steps] = build_module(t_steps)
    return _MODULE_CACHE[t_steps]


# --------------------------------------------------------------------------
# Host-side sharding / gather
# --------------------------------------------------------------------------

def _fp16_pair(a):
    hi = a.astype(np.float16)
    lo = (a - hi.astype(np.float32)).astype(np.float16)
    return hi, lo


def make_in_maps(x, w1, b1, w2, b2, t_steps: int = T):
    x = np.asarray(x, dtype=np.float32)
    w1 = np.asarray(w1, dtype=np.float32)
    b1 = np.asarray(b1, dtype=np.float32)
    w2 = np.asarray(w2, dtype=np.float32)
    b2 = np.asarray(b2, dtype=np.float32)
    tb = t_steps // XB

    w1h, w1l = _fp16_pair(w1.T)                           # [F, H] each
    w1trip = np.zeros((120, H), np.float16)
    w1trip[0:F] = w1h
    w1trip[F : 2 * F] = w1h
    w1trip[2 * F : 3 * F] = w1l

    w2nh, w2nl = _fp16_pair((-0.5 * w2).T)                # [H, C]
    w2qh = np.zeros((H, 32), np.float16)
    w2ql = np.zeros((H, 32), np.float16)
    w2qh[:, :C] = w2nh
    w2ql[:, :C] = w2nl
    # effective -0.5*w2.T the PE uses; bias reconstructs w2 @ spk
    w_eff = w2nh.astype(np.float32) + w2nl.astype(np.float32)
    corr = -w_eff.sum(axis=0) + b2

    bias1 = np.ascontiguousarray(b1[:, None])
    bias2 = np.zeros((128, 1), np.float32)
    for g in range(NG):
        bias2[32 * g : 32 * g + C, 0] = corr

    in_maps = []
    for c in range(NCORES):
        xc = x[c * BL : (c + 1) * BL, :t_steps, :]        # [BL, t, F]
        xt_ = xc.transpose(1, 2, 0)                       # [t, F, BL]
        xh16, xl16 = _fp16_pair(xt_)
        trip = np.concatenate([xh16, xl16, xh16], axis=1)  # [t, 120, BL]
        xqc = (
            trip.reshape(tb, XB, 120, BL)
            .transpose(0, 2, 1, 3)
            .reshape(tb, 120, XB * BL)
        )
        in_maps.append(
            {
                "xq": np.ascontiguousarray(xqc),
                "w1trip": w1trip,
                "w2qh": w2qh,
                "w2ql": w2ql,
                "bias1": bias1,
                "bias2": bias2,
            }
        )
    return in_maps


def postprocess(results, t_steps: int = T):
    """results: list of per-core dicts with 'out' [NG, C, t, BG] raw m2."""
    outs = []
    for c in range(NCORES):
        r = results[c]["out"]                             # [NG, C, t, BG]
        spk = (r > THR).astype(np.float32)
        spk = spk.transpose(2, 0, 3, 1).reshape(t_steps, BL, C)
        outs.append(spk)
    return np.concatenate(outs, axis=1)                   # [t, B, C]


def kernel(x, w1, b1, w2, b2):
    nc = _get_module(T)
    in_maps = make_in_maps(x, w1, b1, w2, b2, T)
    res = run_bass_kernel_spmd(nc, in_maps, core_ids=list(range(NCORES)))
    return postprocess(res.results, T)
